# revision 19
# baseline (speedup 1.0000x reference)
"""Trainium2 Bass kernel for nn_Attention_27358941675773.

Reference computation (per batch b):
    q = x @ Q              [N, H]
    k = x @ K              [N, H]
    V = V_down @ V_up      [L, L]
    v = x @ V              [N, L]
    S = q @ k.T / 256      [N, N]
    out = softmax(S) @ v   [N, L]

Sharding: pure data-parallel over batch B=8 across the 8 NeuronCores
(one batch element per core); small params replicated. No collectives.

Per-core kernel design (N=4096, L=256, H=128):
  - Inputs shipped as fp16 (x transposed to [L, N]); all matmuls run at
    full PE rate. qT [H,N] and kT [H,N] are computed directly in
    transposed layout so scores are built as S_T[m, n] (keys on the
    partition axis) with no transposes anywhere in the pipeline.
  - Value path factored through the rank-H bottleneck:
        out = softmax(S) @ x @ V_down @ V_up
    so the O(N^2) product contracts into H=128 columns and V_up is
    applied after the softmax.
  - exp(S_T/256) runs on the Scalar engine straight out of PSUM in
    [128, 1024] tiles through a 3-slot PSUM ring (the scheduler's WAR
    semaphores release one exp late, so a ring depth of 3 is the
    minimum that keeps QK fully pipelined against the exp stream; wider
    exp tiles are geometrically impossible with 8 PSUM banks and the
    2-bank PV accumulator). The exp stream is the critical path.
  - exp output lands in PAIRED [128, 2048] est tiles (two exps fill one
    tile), letting the rowsum tree run 2048-wide bf16 adds on the
    Vector engine: 16 tree ops per block instead of 31, and the last
    pair folds separately so the post-last-exp chain is short.
  - Partition-axis rowsum reduce+broadcast in ONE matmul with an
    all-ones [128,128] fp32r stationary operand (full PE rate),
    replacing the 8.3us GpSimd PartitionAllReduce.
  - The numerator copy (mid -> SBUF) and normalization ride the Vector
    engine; the Scalar engine runs nothing but the exp stream.
  - DMA: weights first as single strided descriptors on the GpSimd
    queue, then x in 4 large descriptors on the SP queue, so the
    projection weights never queue behind the 2MB x transfer.
  - Uniform half-block-lagged schedule: per key tile the PE runs 2 QK
    matmuls plus 2 lagged attention@w matmuls; block 0 uses the
    projections (batched 4-8 to a PSUM slot) as its filler; junk
    matmuls warm the PE clock gate during the input DMA.
  - Output stored transposed [L, N] fp16; host un-transposes on gather.
"""

import os
import sys

import numpy as np

for _p in ("/opt/trn_rl_repo",):
    if _p not in sys.path and os.path.isdir(_p):
        sys.path.insert(0, _p)

B, N, L, H = 8, 4096, 256, 128
SCALER = 256.0
NB = 1024           # query-block (free dim of score tiles)
NBH = 512           # half block (one PSUM bank of fp32)
NT = N // NB        # 4 query blocks
MT = N // 128       # 32 key tiles of 128
P = 128


def _build():
    import concourse.bass as bass
    import concourse.tile as tile
    from concourse import bacc, bass_isa, mybir
    from contextlib import ExitStack

    import bass_rust as _br

    f32 = mybir.dt.float32
    f32r = mybir.dt.float32r
    f16 = mybir.dt.float16
    bf16 = mybir.dt.bfloat16
    AF = mybir.ActivationFunctionType

    nc = bacc.Bacc(
        "TRN2", target_bir_lowering=False, debug=False, num_devices=B
    )

    xT_ext = nc.declare_dram_parameter("xT", [L, N], f16, isOutput=False)
    wq_ext = nc.declare_dram_parameter("Wq", [L, H], f16, isOutput=False)
    wk_ext = nc.declare_dram_parameter("Wk", [L, H], f16, isOutput=False)
    vd_ext = nc.declare_dram_parameter("Vd", [L, H], f16, isOutput=False)
    vu_ext = nc.declare_dram_parameter("Vu", [H, L], f16, isOutput=False)
    # output stored transposed [L, N]; host un-transposes at gather
    out_ext = nc.declare_dram_parameter("out", [L, N], f16, isOutput=True)

    with tile.TileContext(nc) as tc, ExitStack() as ctx:
        persist = ctx.enter_context(tc.tile_pool(name="persist", bufs=1))

        ones32f = persist.tile([P, P], f32)
        nc.gpsimd.memset(ones32f[:], 1.0)
        ones32 = persist.tile([P, P], f32r)
        nc.vector.tensor_copy(ones32[:], ones32f[:])
        # touch Exp right away so the ~2.7us ACT table load overlaps the
        # input DMAs instead of delaying the first real exp
        dum = persist.tile([1, 2], f32)
        nc.gpsimd.memset(dum[:], 0.0)
        nc.scalar.activation(dum[:, 1:2], dum[:, 0:1], AF.Exp)
        wrm = persist.tile([P, NBH], bf16, name="wrm")
        nc.vector.memset(wrm[:], 0.0)

        qw16 = persist.tile([P, 2 * H], f16)    # Q   [l_chunk][l_in, h]
        kw16 = persist.tile([P, 2 * H], f16)
        vd16 = persist.tile([P, 2 * H], f16)    # V_down [l_chunk][l_in, h]
        vu16 = persist.tile([P, L], f16)        # V_up   [h, l]
        vu_bf = persist.tile([P, L], bf16)      # V_up as bf16 (out matmul)
        xt16 = [persist.tile([P, N], f16, name=f"xt16_{c}") for c in range(2)]
        qT16 = persist.tile([P, N], f16)        # q.T       [h, n]
        kT16 = persist.tile([P, N], f16)        # k.T       [h, m]
        w_sb = persist.tile([P, MT * H], bf16)  # x@V_down  [m_tile][m_in, h]

        # ---------------- phase A: direct fp16 loads ----------------
        # weights first (single strided descriptors on the GpSimd queue)
        # so they never queue behind the 2MB x transfer; x s0 chunks next
        # (critical path for the first QK tiles), then the rest of x.
        def dma_w(w_ext, w_sbuf):
            nc.gpsimd.dma_start(
                w_sbuf[:].rearrange("p (c h) -> p c h", c=2),
                w_ext[:, :].rearrange("(c p) h -> p c h", c=2),
            )
        dma_w(wq_ext, qw16)
        for c in range(2):
            nc.sync.dma_start(
                xt16[c][:, 0:NB], xT_ext[c * P:(c + 1) * P, 0:NB]
            )
        dma_w(wk_ext, kw16)
        dma_w(vd_ext, vd16)
        for c in range(2):
            nc.sync.dma_start(
                xt16[c][:, NB:N], xT_ext[c * P:(c + 1) * P, NB:N]
            )
        nc.gpsimd.dma_start(vu16[:], vu_ext[:, :])

        # ------------- phases B+C: projections fused with attention -------
        with (
            tc.tile_pool(name="est", bufs=20) as est_pool,
            tc.tile_pool(name="tree", bufs=2) as tree_pool,
            tc.tile_pool(name="sb_small", bufs=2) as sb_small,
            tc.tile_pool(name="outfin", bufs=4) as outfin_pool,
            tc.tile_pool(name="stp", bufs=3, space="PSUM") as stp,
            tc.tile_pool(name="mtp", bufs=1, space="PSUM") as mtp,
        ):
            chain = [None]   # name of the previous slot's exp instruction

            def chained(inst):
                # pin background PE work behind the exp stream so the
                # scheduler cannot front-load it into boundary bursts
                if chain[0] is not None:
                    s = _br.InstructionNameOrderedSet()
                    s.add(chain[0])
                    inst.ins.add_sync_dependencies_from(s)
                return inst

            est = {}      # (k, pair) -> bf16 [128, 2048] exp tiles (2 halves)
            mtiles = {}   # k -> psum numerator mid^T [h, n] tile
            mscs = {}     # k -> normalized mid (bf16, SBUF)
            bc = {}       # k -> [128, NB] f32 broadcast 1/rowsum
            tr = {}       # tree tiles by (k, name)

            def est_ap(k, j, h):
                off = (j % 2) * NB + h * NBH
                return est[(k, j // 2)][:, off:off + NBH]

            def proj_qkT_pair(w16, dst, f):
                # projects halves f and f+1 into one psum slot, one copy
                ps = stp.tile([P, NB], f32, tag="stp", name=f"pjp_{f}")
                for half in range(2):
                    ff = f + half
                    for c in range(2):
                        mm = nc.tensor.matmul(
                            ps[:, half * NBH:(half + 1) * NBH],
                            w16[:, c * H:(c + 1) * H],
                            xt16[c][:, ff * NBH:(ff + 1) * NBH],
                            start=(c == 0), stop=(c == 1),
                        )
                        if half == 0 and c == 0:
                            chained(mm)
                nc.vector.tensor_copy(dst[:, f * NBH:(f + 2) * NBH], ps[:])

            def proj_qkT_head(w16, dst, f, on_act):
                # single half with its own copy (prologue: ACT is idle)
                ps = stp.tile([P, NB], f32, tag="stp", name=f"pjh_{f}")
                for c in range(2):
                    nc.tensor.matmul(
                        ps[:, :NBH],
                        w16[:, c * H:(c + 1) * H],
                        xt16[c][:, f * NBH:(f + 1) * NBH],
                        start=(c == 0), stop=(c == 1),
                    )
                if on_act:
                    nc.scalar.activation(
                        dst[:, f * NBH:(f + 1) * NBH], ps[:, :NBH], AF.Copy
                    )
                else:
                    nc.vector.tensor_copy(
                        dst[:, f * NBH:(f + 1) * NBH], ps[:, :NBH]
                    )

            def proj_w_batch(b):
                # w tiles 4b..4b+3 into one psum slot, one copy
                ps = stp.tile([P, NB], f32, tag="stp", name=f"pjw_{b}")
                for j4 in range(4):
                    j = 4 * b + j4
                    for c in range(2):
                        mm = nc.tensor.matmul(
                            ps[:, j4 * H:(j4 + 1) * H],
                            xt16[c][:, j * P:(j + 1) * P],
                            vd16[:, c * H:(c + 1) * H],
                            start=(c == 0), stop=(c == 1),
                        )
                        if j4 == 0 and c == 0:
                            chained(mm)
                nc.vector.tensor_copy(
                    w_sb[:, b * NBH:(b + 1) * NBH], ps[:, :NBH]
                )

            def qk_exp(k, mt):
                # high priority: the exp stream paces the kernel; the
                # scheduler's ACT cost model runs ~10% optimistic and
                # otherwise front-loads PV bursts that starve it
                with tc.high_priority(offset=3000):
                    ps = stp.tile([P, NB], f32, tag="stp",
                                  name=f"qk_{k}_{mt}")
                    for h in range(2):
                        nc.tensor.matmul(
                            ps[:, h * NBH:(h + 1) * NBH],
                            kT16[:, mt * P:(mt + 1) * P],
                            qT16[:, k * NB + h * NBH: k * NB + (h + 1) * NBH],
                            start=True, stop=True,
                        )
                    if mt % 2 == 0:
                        e = est_pool.tile([P, 2 * NB], bf16, tag="est",
                                          name=f"est_{k}_{mt // 2}")
                        est[(k, mt // 2)] = e
                    else:
                        e = est[(k, mt // 2)]
                    ei = nc.scalar.activation(
                        e[:, (mt % 2) * NB:(mt % 2 + 1) * NB], ps[:],
                        AF.Exp, scale=1.0 / SCALER,
                    )
                    chain[0] = ei.ins.name

            def tadd(k, name, a, b, dtype, width, bufs=None):
                tag = name.rstrip("0123456789") or name
                if bufs is None:
                    bufs = {"u": 3, "b": 2}.get(tag, 1)
                t = tree_pool.tile([P, width], dtype, tag=tag, bufs=bufs,
                                   name=f"{name}_{k}")
                nc.vector.tensor_add(t[:], a, b)
                tr[(k, name)] = t
                return t

            def tree_adds(k, mt):
                # 2048-wide pairwise tree over est pairs 0..14; pair 15
                # folds separately at the next block head (short tail)
                W2 = 2 * NB
                if mt % 4 == 3 and mt <= 27:
                    i = mt // 4
                    tadd(k, f"u{i}", est[(k, 2 * i)][:],
                         est[(k, 2 * i + 1)][:], bf16, W2)
                if mt == 7:
                    tadd(k, "b0", tr[(k, "u0")][:], tr[(k, "u1")][:], bf16, W2)
                if mt == 15:
                    tadd(k, "b1", tr[(k, "u2")][:], tr[(k, "u3")][:], bf16, W2)
                    tadd(k, "c0", tr[(k, "b0")][:], tr[(k, "b1")][:], bf16, W2)
                if mt == 23:
                    tadd(k, "d0", tr[(k, "u4")][:], tr[(k, "u5")][:], bf16, W2)
                if mt == 27:
                    tadd(k, "e0", tr[(k, "d0")][:], tr[(k, "u6")][:], bf16, W2)
                if mt == 29:
                    tadd(k, "g0", tr[(k, "e0")][:], est[(k, 14)][:], bf16, W2)
                    tadd(k, "t4", tr[(k, "c0")][:], tr[(k, "g0")][:], bf16, W2)
                if mt == 30:
                    t4 = tr[(k, "t4")]
                    t5p = tree_pool.tile([P, NB], f32, tag="t5p", bufs=2,
                                         name=f"t5p_{k}")
                    nc.vector.tensor_add(t5p[:], t4[:, 0:NB], t4[:, NB:2 * NB])
                    tr[(k, "t5p")] = t5p

            def fold_last(k):
                # fold the final pair (tiles 30,31) into the rowsum
                f15 = tree_pool.tile([P, NB], f32, tag="f15", bufs=1,
                                     name=f"f15_{k}")
                nc.vector.tensor_add(
                    f15[:], est[(k, 15)][:, 0:NB], est[(k, 15)][:, NB:2 * NB]
                )
                t = tree_pool.tile([P, NB], f32r, tag="t5", bufs=1,
                                   name=f"t5_{k}")
                nc.vector.tensor_add(t[:], tr[(k, "t5p")][:], f15[:])
                tr[(k, "t5")] = t

            def bc_chain(k):
                # partition-sum + broadcast in one all-ones fp32r matmul
                ps = stp.tile([P, NB], f32, tag="stp", name=f"bcm_{k}")
                for h in range(2):
                    mm = nc.tensor.matmul(
                        ps[:, h * NBH:(h + 1) * NBH],
                        ones32[:],
                        tr[(k, "t5")][:, h * NBH:(h + 1) * NBH],
                        start=True, stop=True,
                    )
                    if h == 0:
                        chained(mm)
                bck = sb_small.tile([P, NB], f32, tag="bc", bufs=2,
                                    name=f"bc_{k}")
                nc.vector.reciprocal_approx_fast(bck[:], ps[:])
                bc[k] = bck

            def norm_mid(k):
                msc = sb_small.tile([P, NB], bf16, tag="msc", bufs=2,
                                    name=f"msc_{k}")
                nc.vector.tensor_copy(msc[:], mtiles[k][:])
                mscs[k] = msc

            def drain_out(k):
                # apply V_up, normalize by 1/rowsum, store transposed (f16)
                for lt in range(2):
                    op = stp.tile([P, NB], f32, tag="stp", name=f"op_{k}_{lt}")
                    for h in range(2):
                        mm = nc.tensor.matmul(
                            op[:, h * NBH:(h + 1) * NBH],
                            vu_bf[:, lt * P:(lt + 1) * P],
                            mscs[k][:, h * NBH:(h + 1) * NBH],
                            start=True, stop=True,
                        )
                        if h == 0:
                            chained(mm)
                    fin = outfin_pool.tile([P, NB], f16, tag="fin")
                    nc.vector.tensor_mul(fin[:], op[:], bc[k][:])
                    nc.gpsimd.dma_start(
                        out_ext[lt * P:(lt + 1) * P, k * NB:(k + 1) * NB],
                        fin[:],
                    )

            def pv2(kk, j, mid):
                for h in range(2):
                    mm = nc.tensor.matmul(
                        mid[:, h * NBH:(h + 1) * NBH],
                        w_sb[:, j * H:(j + 1) * H],
                        est_ap(kk, j, h),
                        start=(j == 0), stop=(j == MT - 1),
                    )
                    if h == 0:
                        chained(mm)

            # PE warm-up: junk matmuls while the input DMA is in flight
            for i in range(14):
                ps = stp.tile([P, NB], f32, tag="stp", name=f"warm_{i}")
                nc.tensor.matmul(
                    ps[:, :NBH], wrm[:, :P], wrm[:], start=True, stop=True
                )

            # head: the first QK tiles need qT/kT half-blocks 0,1 (s0).
            # kT first: its DVE copies are the critical path to QK(0,0);
            # the q copies ride the idle Scalar engine in parallel.
            proj_qkT_head(kw16, kT16, 0, on_act=False)
            proj_qkT_head(qw16, qT16, 0, on_act=True)
            proj_qkT_head(kw16, kT16, 1, on_act=False)
            proj_qkT_head(qw16, qT16, 1, on_act=True)

            # Uniform half-block-lagged schedule: during block k the PE
            # runs QK(k) plus the oldest pending attention@w work; block 0
            # uses the batched projections as its filler.
            for k in range(NT):
                for mt in range(MT):
                    qk_exp(k, mt)
                    if k == 0:
                        if mt % 4 == 1 and mt <= 13:
                            proj_w_batch(mt // 4 * 2)
                            proj_w_batch(mt // 4 * 2 + 1)
                        if mt in (2, 10, 18):
                            proj_qkT_pair(kw16, kT16, mt // 8 * 2 + 2)
                        if mt == 15:
                            proj_qkT_pair(qw16, qT16, 2)
                        if mt == 19:
                            nc.gpsimd.tensor_copy(vu_bf[:], vu16[:])
                    if k == 1 and mt in (8, 12):
                        proj_qkT_pair(qw16, qT16, (mt - 8) // 2 + 4)
                    if k >= 1 and mt <= 15:
                        pv2(k - 1, 16 + mt, mtiles[k - 1])
                    if mt == 16:
                        mid = mtp.tile([P, NB], f32, tag="mtp",
                                       name=f"mid_{k}")
                        mtiles[k] = mid
                    if mt >= 16:
                        pv2(k, mt - 16, mtiles[k])
                    if k == NT - 1 and mt >= 20:
                        # last block: pull forward part of the epilogue
                        pv2(k, mt - 4, mtiles[k])
                    if k >= 1:
                        if mt == 0:
                            fold_last(k - 1)
                        if mt == 2:
                            bc_chain(k - 1)
                        if mt == 15:
                            norm_mid(k - 1)
                        if mt == 22:
                            drain_out(k - 1)
                    tree_adds(k, mt)

            # epilogue: finish block 3's product and drain it
            k3 = NT - 1
            for j in range(28, MT):
                pv2(k3, j, mtiles[k3])
            fold_last(k3)
            bc_chain(k3)
            norm_mid(k3)
            drain_out(k3)

    if not nc.is_finalized():
        nc.finalize()
    return nc


_GRAPH_CACHE = {}


def _get_graph():
    if "nc" not in _GRAPH_CACHE:
        _GRAPH_CACHE["nc"] = _build()
    return _GRAPH_CACHE["nc"]


def run(inputs: dict, trace: bool = False):
    """Run the SPMD kernel on 8 cores. Returns (output, BassKernelResults)."""
    from concourse.bass_utils import run_bass_kernel_spmd

    x = np.asarray(inputs["x"], dtype=np.float32)
    Q = np.asarray(inputs["Q"], dtype=np.float32)[0]
    K = np.asarray(inputs["K"], dtype=np.float32)[0]
    Vd = np.asarray(inputs["V_down"], dtype=np.float32)[0]
    Vu = np.asarray(inputs["V_up"], dtype=np.float32)[0]

    wq = np.ascontiguousarray(Q).astype(np.float16)
    wk = np.ascontiguousarray(K).astype(np.float16)
    vd = np.ascontiguousarray(Vd).astype(np.float16)
    vu = np.ascontiguousarray(Vu).astype(np.float16)

    in_maps = []
    for b in range(B):
        in_maps.append({
            "xT": np.ascontiguousarray(x[b].T).astype(np.float16),
            "Wq": wq,
            "Wk": wk,
            "Vd": vd,
            "Vu": vu,
        })

    nc = _get_graph()
    res = run_bass_kernel_spmd(nc, in_maps, core_ids=list(range(B)), trace=trace)
    # device output is [L, N] per core; un-transpose during the gather
    out = np.stack([np.asarray(res.results[i]["out"]).astype(np.float32).T for i in range(B)])
    return np.ascontiguousarray(out, dtype=np.float32), res


def kernel(**inputs) -> np.ndarray:
    out, _ = run(inputs, trace=False)
    return out


# revision 21
# speedup vs baseline: 1.2301x; 1.2301x over previous
"""Trainium2 Bass kernel for nn_Attention_27358941675773.

Reference computation (per batch b):
    q = x @ Q              [N, H]
    k = x @ K              [N, H]
    V = V_down @ V_up      [L, L]
    v = x @ V              [N, L]
    S = q @ k.T / 256      [N, N]
    out = softmax(S) @ v   [N, L]

Sharding: pure data-parallel over batch B=8 across the 8 NeuronCores
(one batch element per core); small params replicated. No collectives.

Per-core kernel design (N=4096, L=256, H=128):
  - Inputs shipped as fp16 (x transposed to [L, N]); all matmuls run at
    full PE rate. qT [H,N] and kT [H,N] are computed directly in
    transposed layout so scores are built as S_T[m, n] (keys on the
    partition axis) with no transposes anywhere in the pipeline.
  - Value path factored through the rank-H bottleneck:
        out = softmax(S) @ x @ V_down @ V_up
    so the O(N^2) product contracts into H=128 columns and V_up is
    applied after the softmax.
  - exp(S_T/256) runs on the Scalar engine straight out of PSUM in
    [128, 1024] tiles through a 3-slot PSUM ring (the scheduler's WAR
    semaphores release one exp late, so a ring depth of 3 is the
    minimum that keeps QK fully pipelined against the exp stream; wider
    exp tiles are geometrically impossible with 8 PSUM banks and the
    2-bank PV accumulator). The exp stream is the critical path.
  - exp output lands in PAIRED [128, 2048] est tiles (two exps fill one
    tile), letting the rowsum tree run 2048-wide bf16 adds on the
    Vector engine: 16 tree ops per block instead of 31, and the last
    pair folds separately so the post-last-exp chain is short.
  - Partition-axis rowsum reduce+broadcast in ONE matmul with an
    all-ones [128,128] fp32r stationary operand (full PE rate),
    replacing the 8.3us GpSimd PartitionAllReduce.
  - The numerator copy (mid -> SBUF) and normalization ride the Vector
    engine; the Scalar engine runs nothing but the exp stream.
  - DMA: weights first as single strided descriptors on the GpSimd
    queue, then x in 4 large descriptors on the SP queue, so the
    projection weights never queue behind the 2MB x transfer.
  - Uniform half-block-lagged schedule: per key tile the PE runs 2 QK
    matmuls plus 2 lagged attention@w matmuls; block 0 uses the
    projections (batched 4-8 to a PSUM slot) as its filler; junk
    matmuls warm the PE clock gate during the input DMA.
  - Output stored transposed [L, N] fp16; host un-transposes on gather.
"""

import os
import sys

import numpy as np

for _p in ("/opt/trn_rl_repo",):
    if _p not in sys.path and os.path.isdir(_p):
        sys.path.insert(0, _p)

B, N, L, H = 8, 4096, 256, 128
SCALER = 256.0
NB = 1024           # query-block (free dim of score tiles)
NBH = 512           # half block (one PSUM bank of fp32)
NT = N // NB        # 4 query blocks
MT = N // 128       # 32 key tiles of 128
P = 128


def _build():
    import concourse.bass as bass
    import concourse.tile as tile
    from concourse import bacc, bass_isa, mybir
    from contextlib import ExitStack

    import bass_rust as _br

    f32 = mybir.dt.float32
    f32r = mybir.dt.float32r
    f16 = mybir.dt.float16
    bf16 = mybir.dt.bfloat16
    AF = mybir.ActivationFunctionType

    nc = bacc.Bacc(
        "TRN2", target_bir_lowering=False, debug=False, num_devices=B
    )

    xT_ext = nc.declare_dram_parameter("xT", [L, N], f16, isOutput=False)
    wq_ext = nc.declare_dram_parameter("Wq", [L, H], f16, isOutput=False)
    wk_ext = nc.declare_dram_parameter("Wk", [L, H], f16, isOutput=False)
    vd_ext = nc.declare_dram_parameter("Vd", [L, H], f16, isOutput=False)
    vu_ext = nc.declare_dram_parameter("Vu", [H, L], f16, isOutput=False)
    # output stored transposed [L, N]; host un-transposes at gather
    out_ext = nc.declare_dram_parameter("out", [L, N], f16, isOutput=True)

    with tile.TileContext(nc) as tc, ExitStack() as ctx:
        persist = ctx.enter_context(tc.tile_pool(name="persist", bufs=1))

        ones32f = persist.tile([P, P], f32)
        nc.gpsimd.memset(ones32f[:], 1.0)
        ones32 = persist.tile([P, P], f32r)
        nc.vector.tensor_copy(ones32[:], ones32f[:])
        # touch Exp right away so the ~2.7us ACT table load overlaps the
        # input DMAs instead of delaying the first real exp
        dum = persist.tile([1, 2], f32)
        nc.gpsimd.memset(dum[:], 0.0)
        nc.scalar.activation(dum[:, 1:2], dum[:, 0:1], AF.Exp)
        wrm = persist.tile([P, NBH], bf16, name="wrm")
        nc.vector.memset(wrm[:], 0.0)

        qw16 = persist.tile([P, 2 * H], f16)    # Q   [l_chunk][l_in, h]
        kw16 = persist.tile([P, 2 * H], f16)
        vd16 = persist.tile([P, 2 * H], f16)    # V_down [l_chunk][l_in, h]
        vu16 = persist.tile([P, L], f16)        # V_up   [h, l]
        vu_bf = persist.tile([P, L], bf16)      # V_up as bf16 (out matmul)
        xt16 = [persist.tile([P, N], f16, name=f"xt16_{c}") for c in range(2)]
        qT16 = persist.tile([P, N], f16)        # q.T       [h, n]
        kT16 = persist.tile([P, N], f16)        # k.T       [h, m]
        w_sb = persist.tile([P, MT * H], bf16)  # x@V_down  [m_tile][m_in, h]

        # ---------------- phase A: direct fp16 loads ----------------
        # weights first (single strided descriptors on the GpSimd queue)
        # so they never queue behind the 2MB x transfer; x s0 chunks next
        # (critical path for the first QK tiles), then the rest of x.
        def dma_w(w_ext, w_sbuf):
            nc.gpsimd.dma_start(
                w_sbuf[:].rearrange("p (c h) -> p c h", c=2),
                w_ext[:, :].rearrange("(c p) h -> p c h", c=2),
            )
        dma_w(wq_ext, qw16)
        for c in range(2):
            nc.sync.dma_start(
                xt16[c][:, 0:NB], xT_ext[c * P:(c + 1) * P, 0:NB]
            )
        dma_w(wk_ext, kw16)
        dma_w(vd_ext, vd16)
        for c in range(2):
            nc.sync.dma_start(
                xt16[c][:, NB:N], xT_ext[c * P:(c + 1) * P, NB:N]
            )
        nc.gpsimd.dma_start(vu16[:], vu_ext[:, :])

        # ------------- phases B+C: projections fused with attention -------
        with (
            tc.tile_pool(name="est", bufs=20) as est_pool,
            tc.tile_pool(name="tree", bufs=2) as tree_pool,
            tc.tile_pool(name="sb_small", bufs=2) as sb_small,
            tc.tile_pool(name="outfin", bufs=4) as outfin_pool,
            tc.tile_pool(name="stp", bufs=3, space="PSUM") as stp,
            tc.tile_pool(name="mtp", bufs=1, space="PSUM") as mtp,
        ):
            chain = [None, None]   # exp instruction names: [prev, prev2]

            def chained(inst):
                # pin background PE work behind the exp stream (two slots
                # back) so the scheduler cannot front-load it into
                # boundary bursts, while leaving one slot of slack
                if chain[1] is not None:
                    s = _br.InstructionNameOrderedSet()
                    s.add(chain[1])
                    inst.ins.add_sync_dependencies_from(s)
                return inst

            est = {}      # (k, pair) -> bf16 [128, 2048] exp tiles (2 halves)
            mtiles = {}   # k -> psum numerator mid^T [h, n] tile
            mscs = {}     # k -> normalized mid (bf16, SBUF)
            bc = {}       # k -> [128, NB] f32 broadcast 1/rowsum
            tr = {}       # tree tiles by (k, name)

            def est_ap(k, j, h):
                off = (j % 2) * NB + h * NBH
                return est[(k, j // 2)][:, off:off + NBH]

            def proj_qkT_pair(w16, dst, f):
                # projects halves f and f+1 into one psum slot, one copy
                ps = stp.tile([P, NB], f32, tag="stp", name=f"pjp_{f}")
                for half in range(2):
                    ff = f + half
                    for c in range(2):
                        mm = nc.tensor.matmul(
                            ps[:, half * NBH:(half + 1) * NBH],
                            w16[:, c * H:(c + 1) * H],
                            xt16[c][:, ff * NBH:(ff + 1) * NBH],
                            start=(c == 0), stop=(c == 1),
                        )
                        if half == 0 and c == 0:
                            chained(mm)
                nc.vector.tensor_copy(dst[:, f * NBH:(f + 2) * NBH], ps[:])

            def proj_qkT_head(w16, dst, f, on_act):
                # single half with its own copy (prologue: ACT is idle)
                ps = stp.tile([P, NB], f32, tag="stp", name=f"pjh_{f}")
                for c in range(2):
                    nc.tensor.matmul(
                        ps[:, :NBH],
                        w16[:, c * H:(c + 1) * H],
                        xt16[c][:, f * NBH:(f + 1) * NBH],
                        start=(c == 0), stop=(c == 1),
                    )
                if on_act:
                    nc.scalar.activation(
                        dst[:, f * NBH:(f + 1) * NBH], ps[:, :NBH], AF.Copy
                    )
                else:
                    nc.vector.tensor_copy(
                        dst[:, f * NBH:(f + 1) * NBH], ps[:, :NBH]
                    )

            def proj_w_batch(b):
                # w tiles 4b..4b+3 into one psum slot, one copy
                ps = stp.tile([P, NB], f32, tag="stp", name=f"pjw_{b}")
                for j4 in range(4):
                    j = 4 * b + j4
                    for c in range(2):
                        mm = nc.tensor.matmul(
                            ps[:, j4 * H:(j4 + 1) * H],
                            xt16[c][:, j * P:(j + 1) * P],
                            vd16[:, c * H:(c + 1) * H],
                            start=(c == 0), stop=(c == 1),
                        )
                        if j4 == 0 and c == 0:
                            chained(mm)
                nc.vector.tensor_copy(
                    w_sb[:, b * NBH:(b + 1) * NBH], ps[:, :NBH]
                )

            def qk_exp(k, mt):
                # high priority: the exp stream paces the kernel; the
                # scheduler's ACT cost model runs ~10% optimistic and
                # otherwise front-loads PV bursts that starve it
                with tc.high_priority(offset=3000):
                    ps = stp.tile([P, NB], f32, tag="stp",
                                  name=f"qk_{k}_{mt}")
                    for h in range(2):
                        nc.tensor.matmul(
                            ps[:, h * NBH:(h + 1) * NBH],
                            kT16[:, mt * P:(mt + 1) * P],
                            qT16[:, k * NB + h * NBH: k * NB + (h + 1) * NBH],
                            start=True, stop=True,
                        )
                    if mt % 2 == 0:
                        e = est_pool.tile([P, 2 * NB], bf16, tag="est",
                                          name=f"est_{k}_{mt // 2}")
                        est[(k, mt // 2)] = e
                    else:
                        e = est[(k, mt // 2)]
                    ei = nc.scalar.activation(
                        e[:, (mt % 2) * NB:(mt % 2 + 1) * NB], ps[:],
                        AF.Exp, scale=1.0 / SCALER,
                    )
                    chain[1] = chain[0]
                    chain[0] = ei.ins.name

            def tadd(k, name, a, b, dtype, width, bufs=None):
                tag = name.rstrip("0123456789") or name
                if bufs is None:
                    bufs = {"u": 3, "b": 2}.get(tag, 1)
                t = tree_pool.tile([P, width], dtype, tag=tag, bufs=bufs,
                                   name=f"{name}_{k}")
                nc.vector.tensor_add(t[:], a, b)
                tr[(k, name)] = t
                return t

            def tree_adds(k, mt):
                # 2048-wide pairwise tree over est pairs 0..14; pair 15
                # folds separately at the next block head (short tail)
                W2 = 2 * NB
                if mt % 4 == 3 and mt <= 27:
                    i = mt // 4
                    tadd(k, f"u{i}", est[(k, 2 * i)][:],
                         est[(k, 2 * i + 1)][:], bf16, W2)
                if mt == 7:
                    tadd(k, "b0", tr[(k, "u0")][:], tr[(k, "u1")][:], bf16, W2)
                if mt == 15:
                    tadd(k, "b1", tr[(k, "u2")][:], tr[(k, "u3")][:], bf16, W2)
                    tadd(k, "c0", tr[(k, "b0")][:], tr[(k, "b1")][:], bf16, W2)
                if mt == 23:
                    tadd(k, "d0", tr[(k, "u4")][:], tr[(k, "u5")][:], bf16, W2)
                if mt == 27:
                    tadd(k, "e0", tr[(k, "d0")][:], tr[(k, "u6")][:], bf16, W2)
                if mt == 29:
                    tadd(k, "g0", tr[(k, "e0")][:], est[(k, 14)][:], bf16, W2)
                    tadd(k, "t4", tr[(k, "c0")][:], tr[(k, "g0")][:], bf16, W2)
                if mt == 30:
                    t4 = tr[(k, "t4")]
                    t5p = tree_pool.tile([P, NB], f32, tag="t5p", bufs=2,
                                         name=f"t5p_{k}")
                    nc.vector.tensor_add(t5p[:], t4[:, 0:NB], t4[:, NB:2 * NB])
                    tr[(k, "t5p")] = t5p

            def fold_last(k):
                # fold the final pair (tiles 30,31) into the rowsum
                f15 = tree_pool.tile([P, NB], f32, tag="f15", bufs=1,
                                     name=f"f15_{k}")
                nc.vector.tensor_add(
                    f15[:], est[(k, 15)][:, 0:NB], est[(k, 15)][:, NB:2 * NB]
                )
                t = tree_pool.tile([P, NB], f32r, tag="t5", bufs=1,
                                   name=f"t5_{k}")
                nc.vector.tensor_add(t[:], tr[(k, "t5p")][:], f15[:])
                tr[(k, "t5")] = t

            def bc_chain(k):
                # partition-sum + broadcast in one all-ones fp32r matmul
                ps = stp.tile([P, NB], f32, tag="stp", name=f"bcm_{k}")
                for h in range(2):
                    mm = nc.tensor.matmul(
                        ps[:, h * NBH:(h + 1) * NBH],
                        ones32[:],
                        tr[(k, "t5")][:, h * NBH:(h + 1) * NBH],
                        start=True, stop=True,
                    )
                    if h == 0:
                        chained(mm)
                bck = sb_small.tile([P, NB], f32, tag="bc", bufs=2,
                                    name=f"bc_{k}")
                nc.vector.reciprocal_approx_fast(bck[:], ps[:])
                bc[k] = bck

            def norm_mid(k):
                msc = sb_small.tile([P, NB], bf16, tag="msc", bufs=2,
                                    name=f"msc_{k}")
                nc.vector.tensor_copy(msc[:], mtiles[k][:])
                mscs[k] = msc

            def drain_out(k):
                # apply V_up, normalize by 1/rowsum, store transposed (f16)
                for lt in range(2):
                    op = stp.tile([P, NB], f32, tag="stp", name=f"op_{k}_{lt}")
                    for h in range(2):
                        mm = nc.tensor.matmul(
                            op[:, h * NBH:(h + 1) * NBH],
                            vu_bf[:, lt * P:(lt + 1) * P],
                            mscs[k][:, h * NBH:(h + 1) * NBH],
                            start=True, stop=True,
                        )
                        if h == 0:
                            chained(mm)
                    fin = outfin_pool.tile([P, NB], f16, tag="fin")
                    nc.vector.tensor_mul(fin[:], op[:], bc[k][:])
                    nc.gpsimd.dma_start(
                        out_ext[lt * P:(lt + 1) * P, k * NB:(k + 1) * NB],
                        fin[:],
                    )

            def pv2(kk, j, mid):
                for h in range(2):
                    mm = nc.tensor.matmul(
                        mid[:, h * NBH:(h + 1) * NBH],
                        w_sb[:, j * H:(j + 1) * H],
                        est_ap(kk, j, h),
                        start=(j == 0), stop=(j == MT - 1),
                    )
                    if h == 0:
                        chained(mm)

            # PE warm-up: junk matmuls while the input DMA is in flight
            for i in range(14):
                ps = stp.tile([P, NB], f32, tag="stp", name=f"warm_{i}")
                nc.tensor.matmul(
                    ps[:, :NBH], wrm[:, :P], wrm[:], start=True, stop=True
                )

            # head: the first QK tiles need qT/kT half-blocks 0,1 (s0).
            # kT first: its DVE copies are the critical path to QK(0,0);
            # the q copies ride the idle Scalar engine in parallel.
            proj_qkT_head(kw16, kT16, 0, on_act=False)
            proj_qkT_head(qw16, qT16, 0, on_act=True)
            proj_qkT_head(kw16, kT16, 1, on_act=False)
            proj_qkT_head(qw16, qT16, 1, on_act=True)

            # Uniform half-block-lagged schedule: during block k the PE
            # runs QK(k) plus the oldest pending attention@w work; block 0
            # uses the batched projections as its filler.
            for k in range(NT):
                for mt in range(MT):
                    qk_exp(k, mt)
                    if k == 0:
                        if mt % 4 == 1 and mt <= 13:
                            proj_w_batch(mt // 4 * 2)
                            proj_w_batch(mt // 4 * 2 + 1)
                        if mt in (2, 10, 18):
                            proj_qkT_pair(kw16, kT16, mt // 8 * 2 + 2)
                        if mt == 15:
                            proj_qkT_pair(qw16, qT16, 2)
                        if mt == 19:
                            nc.gpsimd.tensor_copy(vu_bf[:], vu16[:])
                    if k == 1 and mt in (8, 12):
                        proj_qkT_pair(qw16, qT16, (mt - 8) // 2 + 4)
                    if k >= 1 and mt <= 15:
                        pv2(k - 1, 16 + mt, mtiles[k - 1])
                    if mt == 16:
                        mid = mtp.tile([P, NB], f32, tag="mtp",
                                       name=f"mid_{k}")
                        mtiles[k] = mid
                    if mt >= 16:
                        pv2(k, mt - 16, mtiles[k])
                    if k == NT - 1 and mt >= 20:
                        # last block: pull forward part of the epilogue
                        pv2(k, mt - 4, mtiles[k])
                    if k == NT - 1 and mt >= 30:
                        pv2(k, mt - 2, mtiles[k])
                    if k >= 1:
                        if mt == 0:
                            fold_last(k - 1)
                        if mt == 2:
                            bc_chain(k - 1)
                        if mt == 15:
                            norm_mid(k - 1)
                        if mt == 22:
                            drain_out(k - 1)
                    tree_adds(k, mt)

            # epilogue: finish block 3's product and drain it
            k3 = NT - 1
            for j in range(30, MT):
                pv2(k3, j, mtiles[k3])
            fold_last(k3)
            bc_chain(k3)
            norm_mid(k3)
            drain_out(k3)

    if not nc.is_finalized():
        nc.finalize()
    return nc


_GRAPH_CACHE = {}


def _get_graph():
    if "nc" not in _GRAPH_CACHE:
        _GRAPH_CACHE["nc"] = _build()
    return _GRAPH_CACHE["nc"]


def run(inputs: dict, trace: bool = False):
    """Run the SPMD kernel on 8 cores. Returns (output, BassKernelResults)."""
    from concourse.bass_utils import run_bass_kernel_spmd

    x = np.asarray(inputs["x"], dtype=np.float32)
    Q = np.asarray(inputs["Q"], dtype=np.float32)[0]
    K = np.asarray(inputs["K"], dtype=np.float32)[0]
    Vd = np.asarray(inputs["V_down"], dtype=np.float32)[0]
    Vu = np.asarray(inputs["V_up"], dtype=np.float32)[0]

    wq = np.ascontiguousarray(Q).astype(np.float16)
    wk = np.ascontiguousarray(K).astype(np.float16)
    vd = np.ascontiguousarray(Vd).astype(np.float16)
    vu = np.ascontiguousarray(Vu).astype(np.float16)

    in_maps = []
    for b in range(B):
        in_maps.append({
            "xT": np.ascontiguousarray(x[b].T).astype(np.float16),
            "Wq": wq,
            "Wk": wk,
            "Vd": vd,
            "Vu": vu,
        })

    nc = _get_graph()
    res = run_bass_kernel_spmd(nc, in_maps, core_ids=list(range(B)), trace=trace)
    # device output is [L, N] per core; un-transpose during the gather
    out = np.stack([np.asarray(res.results[i]["out"]).astype(np.float32).T for i in range(B)])
    return np.ascontiguousarray(out, dtype=np.float32), res


def kernel(**inputs) -> np.ndarray:
    out, _ = run(inputs, trace=False)
    return out


# revision 22
# speedup vs baseline: 1.2521x; 1.0178x over previous
"""Trainium2 Bass kernel for nn_Attention_27358941675773.

Reference computation (per batch b):
    q = x @ Q              [N, H]
    k = x @ K              [N, H]
    V = V_down @ V_up      [L, L]
    v = x @ V              [N, L]
    S = q @ k.T / 256      [N, N]
    out = softmax(S) @ v   [N, L]

Sharding: pure data-parallel over batch B=8 across the 8 NeuronCores
(one batch element per core); small params replicated. No collectives.

Per-core kernel design (N=4096, L=256, H=128):
  - Inputs shipped as fp16 (x transposed to [L, N]); all matmuls run at
    full PE rate. qT [H,N] and kT [H,N] are computed directly in
    transposed layout so scores are built as S_T[m, n] (keys on the
    partition axis) with no transposes anywhere in the pipeline.
  - Value path factored through the rank-H bottleneck:
        out = softmax(S) @ x @ V_down @ V_up
    so the O(N^2) product contracts into H=128 columns and V_up is
    applied after the softmax.
  - exp(S_T/256) runs on the Scalar engine straight out of PSUM in
    [128, 1024] tiles through a 3-slot PSUM ring (the scheduler's WAR
    semaphores release one exp late, so a ring depth of 3 is the
    minimum that keeps QK fully pipelined against the exp stream; wider
    exp tiles are geometrically impossible with 8 PSUM banks and the
    2-bank PV accumulator). The exp stream is the critical path.
  - exp output lands in PAIRED [128, 2048] est tiles (two exps fill one
    tile), letting the rowsum tree run 2048-wide bf16 adds on the
    Vector engine: 16 tree ops per block instead of 31, and the last
    pair folds separately so the post-last-exp chain is short.
  - Partition-axis rowsum reduce+broadcast in ONE matmul with an
    all-ones [128,128] fp32r stationary operand (full PE rate),
    replacing the 8.3us GpSimd PartitionAllReduce.
  - The numerator copy (mid -> SBUF) and normalization ride the Vector
    engine; the Scalar engine runs nothing but the exp stream.
  - DMA: weights first as single strided descriptors on the GpSimd
    queue, then x in 4 large descriptors on the SP queue, so the
    projection weights never queue behind the 2MB x transfer.
  - Uniform half-block-lagged schedule: per key tile the PE runs 2 QK
    matmuls plus 2 lagged attention@w matmuls; block 0 uses the
    projections (batched 4-8 to a PSUM slot) as its filler; junk
    matmuls warm the PE clock gate during the input DMA.
  - Output stored transposed [L, N] fp16; host un-transposes on gather.
"""

import os
import sys

import numpy as np

for _p in ("/opt/trn_rl_repo",):
    if _p not in sys.path and os.path.isdir(_p):
        sys.path.insert(0, _p)

B, N, L, H = 8, 4096, 256, 128
SCALER = 256.0
NB = 1024           # query-block (free dim of score tiles)
NBH = 512           # half block (one PSUM bank of fp32)
NT = N // NB        # 4 query blocks
MT = N // 128       # 32 key tiles of 128
P = 128


def _build():
    import concourse.bass as bass
    import concourse.tile as tile
    from concourse import bacc, bass_isa, mybir
    from contextlib import ExitStack

    import bass_rust as _br

    f32 = mybir.dt.float32
    f32r = mybir.dt.float32r
    f16 = mybir.dt.float16
    bf16 = mybir.dt.bfloat16
    AF = mybir.ActivationFunctionType

    nc = bacc.Bacc(
        "TRN2", target_bir_lowering=False, debug=False, num_devices=B
    )

    xT_ext = nc.declare_dram_parameter("xT", [L, N], f16, isOutput=False)
    wq_ext = nc.declare_dram_parameter("Wq", [L, H], f16, isOutput=False)
    wk_ext = nc.declare_dram_parameter("Wk", [L, H], f16, isOutput=False)
    vd_ext = nc.declare_dram_parameter("Vd", [L, H], f16, isOutput=False)
    vu_ext = nc.declare_dram_parameter("Vu", [H, L], f16, isOutput=False)
    # output stored transposed [L, N]; host un-transposes at gather
    out_ext = nc.declare_dram_parameter("out", [L, N], f16, isOutput=True)

    with tile.TileContext(nc) as tc, ExitStack() as ctx:
        persist = ctx.enter_context(tc.tile_pool(name="persist", bufs=1))

        ones32f = persist.tile([P, P], f32)
        nc.gpsimd.memset(ones32f[:], 1.0)
        ones32 = persist.tile([P, P], f32r)
        nc.vector.tensor_copy(ones32[:], ones32f[:])
        # touch Exp right away so the ~2.7us ACT table load overlaps the
        # input DMAs instead of delaying the first real exp
        dum = persist.tile([1, 2], f32)
        nc.gpsimd.memset(dum[:], 0.0)
        nc.scalar.activation(dum[:, 1:2], dum[:, 0:1], AF.Exp)
        wrm = persist.tile([P, NBH], bf16, name="wrm")
        nc.vector.memset(wrm[:], 0.0)

        qw16 = persist.tile([P, 2 * H], f16)    # Q   [l_chunk][l_in, h]
        kw16 = persist.tile([P, 2 * H], f16)
        vd16 = persist.tile([P, 2 * H], f16)    # V_down [l_chunk][l_in, h]
        vu16 = persist.tile([P, L], f16)        # V_up   [h, l]
        vu_bf = persist.tile([P, L], bf16)      # V_up as bf16 (out matmul)
        xt16 = [persist.tile([P, N], f16, name=f"xt16_{c}") for c in range(2)]
        qT16 = persist.tile([P, N], f16)        # q.T       [h, n]
        kT16 = persist.tile([P, N], f16)        # k.T       [h, m]
        w_sb = persist.tile([P, MT * H], bf16)  # x@V_down  [m_tile][m_in, h]

        # ---------------- phase A: direct fp16 loads ----------------
        # weights first (single strided descriptors on the GpSimd queue)
        # so they never queue behind the 2MB x transfer; x s0 chunks next
        # (critical path for the first QK tiles), then the rest of x.
        def dma_w(w_ext, w_sbuf):
            nc.gpsimd.dma_start(
                w_sbuf[:].rearrange("p (c h) -> p c h", c=2),
                w_ext[:, :].rearrange("(c p) h -> p c h", c=2),
            )
        for c in range(2):
            nc.sync.dma_start(
                xt16[c][:, 0:NB], xT_ext[c * P:(c + 1) * P, 0:NB]
            )
        dma_w(wq_ext, qw16)
        dma_w(wk_ext, kw16)
        dma_w(vd_ext, vd16)
        for c in range(2):
            nc.sync.dma_start(
                xt16[c][:, NB:N], xT_ext[c * P:(c + 1) * P, NB:N]
            )
        nc.gpsimd.dma_start(vu16[:], vu_ext[:, :])

        # ------------- phases B+C: projections fused with attention -------
        with (
            tc.tile_pool(name="est", bufs=20) as est_pool,
            tc.tile_pool(name="tree", bufs=2) as tree_pool,
            tc.tile_pool(name="sb_small", bufs=2) as sb_small,
            tc.tile_pool(name="outfin", bufs=4) as outfin_pool,
            tc.tile_pool(name="stp", bufs=3, space="PSUM") as stp,
            tc.tile_pool(name="mtp", bufs=1, space="PSUM") as mtp,
        ):
            chain = [None, None]   # exp instruction names: [prev, prev2]

            def chained(inst):
                # pin background PE work behind the exp stream (two slots
                # back) so the scheduler cannot front-load it into
                # boundary bursts, while leaving one slot of slack
                if chain[1] is not None:
                    s = _br.InstructionNameOrderedSet()
                    s.add(chain[1])
                    inst.ins.add_sync_dependencies_from(s)
                return inst

            est = {}      # (k, pair) -> bf16 [128, 2048] exp tiles (2 halves)
            mtiles = {}   # k -> psum numerator mid^T [h, n] tile
            mscs = {}     # k -> normalized mid (bf16, SBUF)
            bc = {}       # k -> [128, NB] f32 broadcast 1/rowsum
            tr = {}       # tree tiles by (k, name)

            def est_ap(k, j, h):
                off = (j % 2) * NB + h * NBH
                return est[(k, j // 2)][:, off:off + NBH]

            def proj_qkT_pair(w16, dst, f):
                # projects halves f and f+1 into one psum slot, one copy
                ps = stp.tile([P, NB], f32, tag="stp", name=f"pjp_{f}")
                for half in range(2):
                    ff = f + half
                    for c in range(2):
                        mm = nc.tensor.matmul(
                            ps[:, half * NBH:(half + 1) * NBH],
                            w16[:, c * H:(c + 1) * H],
                            xt16[c][:, ff * NBH:(ff + 1) * NBH],
                            start=(c == 0), stop=(c == 1),
                        )
                        if half == 0 and c == 0:
                            chained(mm)
                nc.vector.tensor_copy(dst[:, f * NBH:(f + 2) * NBH], ps[:])

            def proj_qkT_head(w16, dst, f, on_act):
                # single half with its own copy (prologue: ACT is idle)
                ps = stp.tile([P, NB], f32, tag="stp", name=f"pjh_{f}")
                for c in range(2):
                    nc.tensor.matmul(
                        ps[:, :NBH],
                        w16[:, c * H:(c + 1) * H],
                        xt16[c][:, f * NBH:(f + 1) * NBH],
                        start=(c == 0), stop=(c == 1),
                    )
                if on_act:
                    nc.scalar.activation(
                        dst[:, f * NBH:(f + 1) * NBH], ps[:, :NBH], AF.Copy
                    )
                else:
                    nc.vector.tensor_copy(
                        dst[:, f * NBH:(f + 1) * NBH], ps[:, :NBH]
                    )

            def proj_w_batch(b):
                # w tiles 4b..4b+3 into one psum slot, one copy
                ps = stp.tile([P, NB], f32, tag="stp", name=f"pjw_{b}")
                for j4 in range(4):
                    j = 4 * b + j4
                    for c in range(2):
                        mm = nc.tensor.matmul(
                            ps[:, j4 * H:(j4 + 1) * H],
                            xt16[c][:, j * P:(j + 1) * P],
                            vd16[:, c * H:(c + 1) * H],
                            start=(c == 0), stop=(c == 1),
                        )
                        if j4 == 0 and c == 0:
                            chained(mm)
                nc.vector.tensor_copy(
                    w_sb[:, b * NBH:(b + 1) * NBH], ps[:, :NBH]
                )

            def qk_exp(k, mt):
                # high priority: the exp stream paces the kernel; the
                # scheduler's ACT cost model runs ~10% optimistic and
                # otherwise front-loads PV bursts that starve it
                with tc.high_priority(offset=3000):
                    ps = stp.tile([P, NB], f32, tag="stp",
                                  name=f"qk_{k}_{mt}")
                    for h in range(2):
                        nc.tensor.matmul(
                            ps[:, h * NBH:(h + 1) * NBH],
                            kT16[:, mt * P:(mt + 1) * P],
                            qT16[:, k * NB + h * NBH: k * NB + (h + 1) * NBH],
                            start=True, stop=True,
                        )
                    if mt % 2 == 0:
                        e = est_pool.tile([P, 2 * NB], bf16, tag="est",
                                          name=f"est_{k}_{mt // 2}")
                        est[(k, mt // 2)] = e
                    else:
                        e = est[(k, mt // 2)]
                    ei = nc.scalar.activation(
                        e[:, (mt % 2) * NB:(mt % 2 + 1) * NB], ps[:],
                        AF.Exp, scale=1.0 / SCALER,
                    )
                    chain[1] = chain[0]
                    chain[0] = ei.ins.name

            def tadd(k, name, a, b, dtype, width, bufs=None):
                tag = name.rstrip("0123456789") or name
                if bufs is None:
                    bufs = {"u": 3, "b": 2}.get(tag, 1)
                t = tree_pool.tile([P, width], dtype, tag=tag, bufs=bufs,
                                   name=f"{name}_{k}")
                nc.vector.tensor_add(t[:], a, b)
                tr[(k, name)] = t
                return t

            def tree_adds(k, mt):
                # 2048-wide pairwise tree over est pairs 0..14; pair 15
                # folds separately at the next block head (short tail)
                W2 = 2 * NB
                if mt % 4 == 3 and mt <= 27:
                    i = mt // 4
                    tadd(k, f"u{i}", est[(k, 2 * i)][:],
                         est[(k, 2 * i + 1)][:], bf16, W2)
                if mt == 7:
                    tadd(k, "b0", tr[(k, "u0")][:], tr[(k, "u1")][:], bf16, W2)
                if mt == 15:
                    tadd(k, "b1", tr[(k, "u2")][:], tr[(k, "u3")][:], bf16, W2)
                    tadd(k, "c0", tr[(k, "b0")][:], tr[(k, "b1")][:], bf16, W2)
                if mt == 23:
                    tadd(k, "d0", tr[(k, "u4")][:], tr[(k, "u5")][:], bf16, W2)
                if mt == 27:
                    tadd(k, "e0", tr[(k, "d0")][:], tr[(k, "u6")][:], bf16, W2)
                if mt == 29:
                    tadd(k, "g0", tr[(k, "e0")][:], est[(k, 14)][:], bf16, W2)
                    tadd(k, "t4", tr[(k, "c0")][:], tr[(k, "g0")][:], bf16, W2)
                if mt == 30:
                    t4 = tr[(k, "t4")]
                    t5p = tree_pool.tile([P, NB], f32, tag="t5p", bufs=2,
                                         name=f"t5p_{k}")
                    nc.vector.tensor_add(t5p[:], t4[:, 0:NB], t4[:, NB:2 * NB])
                    tr[(k, "t5p")] = t5p

            def fold_last(k):
                # fold the final pair (tiles 30,31) into the rowsum
                f15 = tree_pool.tile([P, NB], f32, tag="f15", bufs=1,
                                     name=f"f15_{k}")
                nc.vector.tensor_add(
                    f15[:], est[(k, 15)][:, 0:NB], est[(k, 15)][:, NB:2 * NB]
                )
                t = tree_pool.tile([P, NB], f32r, tag="t5", bufs=1,
                                   name=f"t5_{k}")
                nc.vector.tensor_add(t[:], tr[(k, "t5p")][:], f15[:])
                tr[(k, "t5")] = t

            def bc_chain(k):
                # partition-sum + broadcast in one all-ones fp32r matmul
                ps = stp.tile([P, NB], f32, tag="stp", name=f"bcm_{k}")
                for h in range(2):
                    mm = nc.tensor.matmul(
                        ps[:, h * NBH:(h + 1) * NBH],
                        ones32[:],
                        tr[(k, "t5")][:, h * NBH:(h + 1) * NBH],
                        start=True, stop=True,
                    )
                    if h == 0:
                        chained(mm)
                bck = sb_small.tile([P, NB], f32, tag="bc", bufs=2,
                                    name=f"bc_{k}")
                nc.vector.reciprocal_approx_fast(bck[:], ps[:])
                bc[k] = bck

            def norm_mid(k):
                msc = sb_small.tile([P, NB], bf16, tag="msc", bufs=2,
                                    name=f"msc_{k}")
                nc.vector.tensor_copy(msc[:], mtiles[k][:])
                mscs[k] = msc

            def drain_out(k):
                # apply V_up, normalize by 1/rowsum, store transposed (f16)
                for lt in range(2):
                    op = stp.tile([P, NB], f32, tag="stp", name=f"op_{k}_{lt}")
                    for h in range(2):
                        mm = nc.tensor.matmul(
                            op[:, h * NBH:(h + 1) * NBH],
                            vu_bf[:, lt * P:(lt + 1) * P],
                            mscs[k][:, h * NBH:(h + 1) * NBH],
                            start=True, stop=True,
                        )
                        if h == 0:
                            chained(mm)
                    fin = outfin_pool.tile([P, NB], f16, tag="fin")
                    nc.vector.tensor_mul(fin[:], op[:], bc[k][:])
                    nc.gpsimd.dma_start(
                        out_ext[lt * P:(lt + 1) * P, k * NB:(k + 1) * NB],
                        fin[:],
                    )

            def pv2(kk, j, mid):
                for h in range(2):
                    mm = nc.tensor.matmul(
                        mid[:, h * NBH:(h + 1) * NBH],
                        w_sb[:, j * H:(j + 1) * H],
                        est_ap(kk, j, h),
                        start=(j == 0), stop=(j == MT - 1),
                    )
                    if h == 0:
                        chained(mm)

            # PE warm-up: junk matmuls while the input DMA is in flight
            # (enough of them to keep the HAM clock gate hot until the
            # first x chunk lands)
            for i in range(24):
                ps = stp.tile([P, NB], f32, tag="stp", name=f"warm_{i}")
                nc.tensor.matmul(
                    ps[:, :NBH], wrm[:, :P], wrm[:], start=True, stop=True
                )

            # head: the first QK tiles need qT/kT half-blocks 0,1 (s0).
            # kT first: its DVE copies are the critical path to QK(0,0);
            # the q copies ride the idle Scalar engine in parallel.
            proj_qkT_head(kw16, kT16, 0, on_act=False)
            proj_qkT_head(qw16, qT16, 0, on_act=True)
            proj_qkT_head(kw16, kT16, 1, on_act=False)
            proj_qkT_head(qw16, qT16, 1, on_act=True)

            # Uniform half-block-lagged schedule: during block k the PE
            # runs QK(k) plus the oldest pending attention@w work; block 0
            # uses the batched projections as its filler.
            for k in range(NT):
                for mt in range(MT):
                    qk_exp(k, mt)
                    if k == 0:
                        if mt % 4 == 1 and mt <= 13:
                            proj_w_batch(mt // 4 * 2)
                            proj_w_batch(mt // 4 * 2 + 1)
                        if mt in (2, 10, 18):
                            proj_qkT_pair(kw16, kT16, mt // 8 * 2 + 2)
                        if mt == 15:
                            proj_qkT_pair(qw16, qT16, 2)
                        if mt == 19:
                            nc.gpsimd.tensor_copy(vu_bf[:], vu16[:])
                    if k == 1 and mt in (8, 12):
                        proj_qkT_pair(qw16, qT16, (mt - 8) // 2 + 4)
                    if k >= 1 and mt <= 15:
                        pv2(k - 1, 16 + mt, mtiles[k - 1])
                    if mt == 16:
                        mid = mtp.tile([P, NB], f32, tag="mtp",
                                       name=f"mid_{k}")
                        mtiles[k] = mid
                    if mt >= 16:
                        pv2(k, mt - 16, mtiles[k])
                    if k == NT - 1 and mt >= 20:
                        # last block: pull forward part of the epilogue
                        pv2(k, mt - 4, mtiles[k])
                    if k == NT - 1 and mt >= 30:
                        pv2(k, mt - 2, mtiles[k])
                    if k >= 1:
                        if mt == 0:
                            fold_last(k - 1)
                        if mt == 2:
                            bc_chain(k - 1)
                        if mt == 15:
                            norm_mid(k - 1)
                        if mt == 22:
                            drain_out(k - 1)
                    tree_adds(k, mt)

            # epilogue: finish block 3's product and drain it
            k3 = NT - 1
            for j in range(30, MT):
                pv2(k3, j, mtiles[k3])
            fold_last(k3)
            bc_chain(k3)
            norm_mid(k3)
            drain_out(k3)

    if not nc.is_finalized():
        nc.finalize()
    return nc


_GRAPH_CACHE = {}


def _get_graph():
    if "nc" not in _GRAPH_CACHE:
        _GRAPH_CACHE["nc"] = _build()
    return _GRAPH_CACHE["nc"]


def run(inputs: dict, trace: bool = False):
    """Run the SPMD kernel on 8 cores. Returns (output, BassKernelResults)."""
    from concourse.bass_utils import run_bass_kernel_spmd

    x = np.asarray(inputs["x"], dtype=np.float32)
    Q = np.asarray(inputs["Q"], dtype=np.float32)[0]
    K = np.asarray(inputs["K"], dtype=np.float32)[0]
    Vd = np.asarray(inputs["V_down"], dtype=np.float32)[0]
    Vu = np.asarray(inputs["V_up"], dtype=np.float32)[0]

    wq = np.ascontiguousarray(Q).astype(np.float16)
    wk = np.ascontiguousarray(K).astype(np.float16)
    vd = np.ascontiguousarray(Vd).astype(np.float16)
    vu = np.ascontiguousarray(Vu).astype(np.float16)

    in_maps = []
    for b in range(B):
        in_maps.append({
            "xT": np.ascontiguousarray(x[b].T).astype(np.float16),
            "Wq": wq,
            "Wk": wk,
            "Vd": vd,
            "Vu": vu,
        })

    nc = _get_graph()
    res = run_bass_kernel_spmd(nc, in_maps, core_ids=list(range(B)), trace=trace)
    # device output is [L, N] per core; un-transpose during the gather
    out = np.stack([np.asarray(res.results[i]["out"]).astype(np.float32).T for i in range(B)])
    return np.ascontiguousarray(out, dtype=np.float32), res


def kernel(**inputs) -> np.ndarray:
    out, _ = run(inputs, trace=False)
    return out
